# revision 9
# baseline (speedup 1.0000x reference)
"""8-bit ripple-carry adder on {0,1} floats — Trainium2 Bass kernel.

Problem: A, B [N=2^23, 8] f32 bits (MSB first), Cin [N,1] f32.
reference ripples from bit 7 (LSB) to bit 0 (MSB):
    t = a + b + c ; s = t mod 2 ; c' = t >= 2
Returns (sums [N,8], carry [N,1]) like the reference.

Sharding: batch dim N split evenly across 8 NeuronCores, no communication.

Host side packs A|B row-wise into one [NS,16] tensor per core so each chunk
is a single DMA (the TT ISA slot supports only one sync wait; two separate
loads would land on two DMAHW lanes and need two waits on the consumer).

Per-core layout: rows are processed in chunks of 128*R rows. A chunk of AB
loads contiguously into an SBUF tile [128, 16R] (partition p holds R full
rows). Bit i of each row: A_i = tile[:, i::16], B_i = tile[:, 8+i::16].

Per bit (DVE = vector, ACT = scalar engine):
    ts = A_i + B_i            (DVE tensor_tensor, strided reads)
    t2 = ts + carry           (DVE tensor_tensor, compact)
    carry = t2 >= 2           (DVE tensor_scalar)
    s_i = |sin(pi/2 * t2)|    (ACT sin then abs; exact-enough on {0,1,2,3})
"""

import math
import os

import numpy as np

N_TOTAL = 8388608
N_CORES = 8
NS = N_TOTAL // N_CORES  # rows per core

F32_R = 512  # rows per partition per chunk (f32 path)

_CACHE = {}


def _build_f32(R: int):
    """Wait-slot-safe pipeline (HW compute instructions fit ONE sync wait).

    Tricks:
      - A|B packed host-side into one [NS,16] tensor -> one load DMA/chunk.
      - bf16 intermediates unlock DVE 2x/4x perf modes (values in {0..3}
        are exact in bf16); I/O stays f32.
      - "primer" ops (tiny memset / 16-col ACT copy) absorb WAR-vs-store
        and WAR-vs-other-engine waits so every real op carries only its
        RAW wait. Engine-level sem observation then covers later ops.
    """
    import concourse.tile as tile
    from concourse import bacc, mybir

    f32 = mybir.dt.float32
    bf16 = mybir.dt.bfloat16
    chunk_rows = 128 * R
    n_chunks = NS // chunk_rows
    assert NS % chunk_rows == 0

    nc = bacc.Bacc(None)
    AB = nc.declare_dram_parameter("AB", [NS, 16], f32, isOutput=False)
    Cin = nc.declare_dram_parameter("Cin", [NS, 1], f32, isOutput=False)
    S = nc.declare_dram_parameter("sums", [NS, 8], f32, isOutput=True)
    CO = nc.declare_dram_parameter("carry", [NS, 1], f32, isOutput=True)

    ABv = AB[:].rearrange("(c p r) m -> c p (r m)", p=128, r=R)
    Cv = Cin[:].rearrange("(c p r) m -> c p (r m)", p=128, r=R)
    Sv = S[:].rearrange("(c p r) m -> c p (r m)", p=128, r=R)
    COv = CO[:].rearrange("(c p r) m -> c p (r m)", p=128, r=R)

    HALF_PI = math.pi / 2.0
    Sin = mybir.ActivationFunctionType.Sin
    Abs = mybir.ActivationFunctionType.Abs
    is_ge = mybir.AluOpType.is_ge

    with tile.TileContext(nc) as tc:
        with (
            tc.tile_pool(name="const", bufs=1) as const_pool,
            tc.tile_pool(name="io", bufs=2) as io_pool,
            tc.tile_pool(name="tmp", bufs=3) as tmp_pool,
        ):
            z16 = const_pool.tile([128, 16], f32, tag="z16")
            nc.vector.memset(z16[:], 0.0)
            # Sin bias: shift inputs {0..3} to [-pi, pi/2] where the ACT
            # spline is accurate (sin(3*pi/2) evaluates to 0.9248 raw).
            npi = const_pool.tile([128, 1], f32, tag="npi")
            nc.vector.memset(npi[:], -math.pi)

            for c in range(n_chunks):
                tAB = io_pool.tile([128, 16 * R], f32, tag="AB")
                nc.sync.dma_start(out=tAB[:], in_=ABv[c])
                tC = io_pool.tile([128, R], f32, tag="Cin")
                nc.sync.dma_start(out=tC[:], in_=Cv[c])
                tOUT = io_pool.tile([128, 8 * R], f32, tag="OUT")

                # ACT-side primer: absorbs tOUT's WAR-vs-sums-store wait and
                # WAW-overlaps every abs write region (cols 0..15 hit every
                # i::8 slice).
                nc.scalar.copy(tOUT[:][:, 0:16], z16[:])

                # t2 segments live in one tile; one strided memset absorbs
                # the WAR-vs-sin (ACT) wait for all 8 segments.
                t2a = tmp_pool.tile([128, 8 * R], bf16, tag="t2")
                nc.vector.memset(t2a[:][:, 0 :: R], 0.0)

                # final-carry slot: primer absorbs WAR-vs-carry-store.
                cnf = tmp_pool.tile([128, R], f32, tag="cnf")
                nc.vector.memset(cnf[:][:, 0:1], 0.0)

                # Real data dep on the Cin DMA (carries its DMA wait).
                carry = tmp_pool.tile([128, R], bf16, tag="c0")
                nc.vector.tensor_copy(carry[:], tC[:])

                for k, i in enumerate([7, 6, 5, 4, 3, 2, 1, 0]):
                    ts = tmp_pool.tile([128, R], bf16, tag="ts")
                    nc.vector.tensor_add(
                        ts[:], tAB[:][:, i::16], tAB[:][:, 8 + i :: 16]
                    )
                    t2 = t2a[:][:, k * R : (k + 1) * R]
                    nc.vector.tensor_add(t2, ts[:], carry[:])
                    if i > 0:
                        cn = tmp_pool.tile([128, R], bf16, tag="cn")
                    else:
                        cn = cnf
                    nc.vector.tensor_scalar(cn[:], t2, 2.0, None, is_ge)
                    sr = tmp_pool.tile([128, R], bf16, tag="sr")
                    nc.scalar.activation(
                        sr[:], t2, Sin, scale=HALF_PI, bias=npi[:]
                    )
                    nc.scalar.activation(tOUT[:][:, i::8], sr[:], Abs)
                    carry = cn

                nc.sync.dma_start(out=Sv[c], in_=tOUT[:])
                nc.sync.dma_start(out=COv[c], in_=cnf[:])
    nc.finalize()
    return nc


def _get_nc():
    key = ("f32", F32_R)
    if key not in _CACHE:
        _CACHE[key] = _build_f32(F32_R)
    return _CACHE[key]


def kernel(A, B, Cin, _trace=False):
    from concourse.bass_utils import run_bass_kernel_spmd

    A = np.asarray(A, dtype=np.float32)
    B = np.asarray(B, dtype=np.float32)
    Cin = np.ascontiguousarray(np.asarray(Cin, dtype=np.float32))
    assert A.shape == (N_TOTAL, 8) and B.shape == (N_TOTAL, 8)
    assert Cin.shape == (N_TOTAL, 1)

    AB = np.empty((N_TOTAL, 16), dtype=np.float32)
    AB[:, :8] = A
    AB[:, 8:] = B

    nc = _get_nc()

    in_maps = []
    for i in range(N_CORES):
        lo, hi = i * NS, (i + 1) * NS
        in_maps.append({"AB": AB[lo:hi], "Cin": Cin[lo:hi]})

    res = run_bass_kernel_spmd(
        nc, in_maps, core_ids=list(range(N_CORES)), trace=_trace
    )

    sums = np.empty((N_TOTAL, 8), dtype=np.float32)
    carry = np.empty((N_TOTAL, 1), dtype=np.float32)
    for i in range(N_CORES):
        lo, hi = i * NS, (i + 1) * NS
        sums[lo:hi] = res.results[i]["sums"]
        carry[lo:hi] = res.results[i]["carry"]

    if _trace:
        kernel.last_exec_time_ns = res.exec_time_ns
    return sums, carry


kernel.last_exec_time_ns = None
